# revision 27
# baseline (speedup 1.0000x reference)
"""NeRF volumetric alpha-compositing kernel for Trainium2 (Bass/Tile).

Full inputs:  rgbo [131072, 128, 4] f32, depth [131072, 128] f32.
Full output:  [131072, 3] f32.

Sharding: data-parallel over rays, 8 cores x 16384 rays.

Per-core layout: ray-per-partition; each superblock covers BLOCK=128
partitions x t_b rays per partition (t_b blocks of S=128 samples on the
free dim).  Per superblock:

  delta[s]  = depth[s+1] - depth[s]                  (GPSIMD)
  m[s]      = opacity[s] * delta[s]; m[S-1] = o*1e9  (GPSIMD)
  cs        = per-ray inclusive cumsum of m          (one DVE scan with a
              0/1 reset pattern: state = A*state + m)
  te[t,0]=1; te[t,1+s] = exp(-cs[t,s])               (ScalarE Exp, fp32)
  w[i]      = te[i] - te[i+1]   -> bf16              (DVE; = T_i * alpha_i)
  g_c       = sigmoid(rgb_c)    -> bf16              (ScalarE)
  out[t,c]  = sum_s w * g_c                          (DVE stt+accum, bf16 2x)

The last-sample FAR_DELTA=1e9 is exact: te[t,S]=exp(-cs[S-1]) underflows
to 0 whenever opacity[S-1] > ~1e-7, else matches the reference.

Measured on HW (loop_iters slope method): 162.8 us/iter vs 172.8 us
baseline and a 113.8 us DMA-only floor (42 MB/core at ~370 GB/s).
Keeping delta/m on DVE (not GPSIMD), bf16 for w/g/stt, the merged
reset-scan, and depth+out DMA on the ACT queue were each verified wins
or neutral; GPSIMD compute, software-pipelined emission, bufs=3, and
the tanh table-reload trick all measured slower on HW.
"""

from contextlib import ExitStack

import numpy as np

import concourse.bass as bass
import concourse.tile as tile
from concourse import bacc, mybir
from concourse.bass_utils import run_bass_kernel_spmd

N_RAYS = 131072
S = 128
N_CORES = 8
NC_RAYS = N_RAYS // N_CORES  # 16384 rays per core
BLOCK = 128                  # rays per partition-block
F32 = mybir.dt.float32
BF16 = mybir.dt.bfloat16


def build_nerf_v2(
    n_rays: int = NC_RAYS,
    t_blocks: int = 8,
    use_bf16: bool = True,
    merged_scan: bool = True,
    gpsimd_delta_m: bool = False,
    gpsimd_channels: int = 0,
    flat_dma: bool = False,
    depth_engine: str = "scalar",
    out_engine: str = "scalar",
    use_tanh: bool = False,
    pipelined: bool = False,
    t_schedule: tuple | None = None,
    repeat: int = 1,
    loop_iters: int = 0,
    skip: tuple = (),
    bufs: int = 2,
    scr_bufs: int = 4,
) -> bass.Bass:
    T = t_blocks
    SUPER = BLOCK * T
    n_super = n_rays // SUPER
    U = S + 1  # te-table stride: 129 entries (T_0..T_128)
    wg_dt = BF16 if use_bf16 else F32

    nc = bacc.Bacc("TRN2", target_bir_lowering=False, debug=False)
    rgbo_h = nc.declare_dram_parameter("rgbo", [n_rays, S, 4], F32, isOutput=False)
    depth_h = nc.declare_dram_parameter("depth", [n_rays, S], F32, isOutput=False)
    out_h = nc.declare_dram_parameter("out", [n_rays, 3], F32, isOutput=True)

    rgbo_ap = rgbo_h.ap()
    depth_ap = depth_h.ap()
    out_ap = out_h.ap()

    def eng_of(name):
        return {"sync": nc.sync, "scalar": nc.scalar, "gpsimd": nc.gpsimd}[name]

    with ExitStack() as ctx:
        tc = ctx.enter_context(tile.TileContext(nc))
        p_rgbo = ctx.enter_context(tc.tile_pool(name="rgbo", bufs=bufs))
        p_depth = ctx.enter_context(tc.tile_pool(name="depth", bufs=bufs))
        p_g = ctx.enter_context(tc.tile_pool(name="g", bufs=bufs))
        p_mid = ctx.enter_context(tc.tile_pool(name="mid", bufs=bufs))
        p_scr = ctx.enter_context(tc.tile_pool(name="scr", bufs=scr_bufs))
        p_out = ctx.enter_context(tc.tile_pool(name="outp", bufs=bufs))
        p_const = ctx.enter_context(tc.tile_pool(name="const", bufs=1))

        # reset pattern for the merged scan: 0.0 at each ray's first
        # sample, 1.0 elsewhere  (state = A*state + m)
        a_t = None
        if merged_scan:
            a_t = p_const.tile([BLOCK, S * T], F32, tag="a")
            nc.vector.memset(a_t[:], 1.0)
            nc.vector.memset(
                a_t.rearrange("p (t s) -> p t s", t=T)[:, :, 0:1], 0.0
            )

        def emit_dma(r0, t_b):
            """Issue the input DMAs for one superblock; returns tile ctx."""
            sup = BLOCK * t_b
            rgbo_t = p_rgbo.tile([BLOCK, 4 * S * t_b], F32, tag="rgbo")
            depth_t = p_depth.tile([BLOCK, S * t_b], F32, tag="depth")
            if flat_dma:
                # partition p holds rays [p*t_b, (p+1)*t_b), contiguous in
                # DRAM -> one descriptor line per partition
                rgbo_dst = rgbo_t[:]
                rgbo_src = rgbo_ap[r0 : r0 + sup].rearrange(
                    "(p t) s c -> p (t s c)", p=BLOCK
                )
                depth_dst = depth_t[:]
                depth_src = depth_ap[r0 : r0 + sup].rearrange(
                    "(p t) s -> p (t s)", p=BLOCK
                )
            else:
                rgbo_dst = rgbo_t.rearrange("p (t f) -> p t f", t=t_b)
                rgbo_src = rgbo_ap[r0 : r0 + sup].rearrange(
                    "(p t) s c -> p t (s c)", p=BLOCK
                )
                depth_dst = depth_t.rearrange("p (t s) -> p t s", t=t_b)
                depth_src = depth_ap[r0 : r0 + sup].rearrange(
                    "(p t) s -> p t s", p=BLOCK
                )
            nc.sync.dma_start(out=rgbo_dst, in_=rgbo_src)
            eng_of(depth_engine).dma_start(out=depth_dst, in_=depth_src)
            return {"r0": r0, "t_b": t_b, "rgbo_t": rgbo_t, "depth_t": depth_t}

        def emit_compute_a(ctx_sb):
            """delta/m/scan/exp/tanh for one superblock (extends ctx)."""
            r0, t_b = ctx_sb["r0"], ctx_sb["t_b"]
            rgbo_t, depth_t = ctx_sb["rgbo_t"], ctx_sb["depth_t"]
            rgbo4 = rgbo_t.rearrange("p (t s c) -> p t s c", t=t_b, s=S, c=4)
            depth3 = depth_t.rearrange("p (t s) -> p t s", t=t_b)

            # per-channel color nonlinearity -> dense bf16 tiles (ScalarE)
            if "sigmoid" in skip:
                g_views = [rgbo4[:, :, :, c] for c in range(3)]
            else:
                g_views = []
                for c in range(3):
                    g_c = p_g.tile([BLOCK, S * t_b], wg_dt, tag=f"g{c}")
                    nc.scalar.activation(
                        g_c.rearrange("p (t s) -> p t s", t=t_b),
                        rgbo4[:, :, :, c],
                        mybir.ActivationFunctionType.Tanh
                        if use_tanh
                        else mybir.ActivationFunctionType.Sigmoid,
                        scale=0.5 if use_tanh else 1.0,
                    )
                    g_views.append(g_c.rearrange("p (t s) -> p t s", t=t_b))

            # delta, m on GPSIMD (or DVE)
            eng_dm = nc.gpsimd if gpsimd_delta_m else nc.vector
            if "dm" in skip:
                m_t = depth_t
            else:
                delta_t = p_mid.tile([BLOCK, S * t_b], F32, tag="delta")
                delta3 = delta_t.rearrange("p (t s) -> p t s", t=t_b)
                eng_dm.tensor_sub(
                    delta3[:, :, 0 : S - 1],
                    depth3[:, :, 1:S],
                    depth3[:, :, 0 : S - 1],
                )
                m_t = p_mid.tile([BLOCK, S * t_b], F32, tag="m")
                m3 = m_t.rearrange("p (t s) -> p t s", t=t_b)
                eng_dm.tensor_mul(
                    m3[:, :, 0 : S - 1],
                    delta3[:, :, 0 : S - 1],
                    rgbo4[:, :, 0 : S - 1, 3],
                )
                eng_dm.tensor_scalar_mul(
                    m3[:, :, S - 1], rgbo4[:, :, S - 1, 3], 1.0e9
                )

            # per-ray inclusive cumsum (DVE scan)
            if "scan" in skip:
                cs_t = m_t
            elif merged_scan:
                cs_t = p_mid.tile([BLOCK, S * t_b], F32, tag="cs")
                nc.vector.tensor_tensor_scan(
                    cs_t[:],
                    a_t[:, 0 : S * t_b],
                    m_t[:],
                    0.0,
                    mybir.AluOpType.mult,
                    mybir.AluOpType.add,
                )
            else:
                cs_t = p_mid.tile([BLOCK, S * t_b], F32, tag="cs")
                for t in range(t_b):
                    nc.vector.tensor_tensor_scan(
                        cs_t[:, t * S : (t + 1) * S],
                        m_t[:, t * S : (t + 1) * S],
                        m_t[:, t * S : (t + 1) * S],
                        0.0,
                        mybir.AluOpType.add,
                        mybir.AluOpType.bypass,
                    )

            # te table (fp32): te[t,0]=1, te[t,1+s]=exp(-cs[t,s])
            te_t = p_mid.tile([BLOCK, U * t_b], F32, tag="te")
            te3 = te_t.rearrange("p (t u) -> p t u", t=t_b)
            nc.vector.memset(te3[:, :, 0:1], 1.0)
            nc.scalar.activation(
                te3[:, :, 1 : S + 1],
                cs_t.rearrange("p (t s) -> p t s", t=t_b),
                mybir.ActivationFunctionType.Exp,
                scale=-1.0,
            )
            ctx_sb["te_t"] = te_t
            ctx_sb["g_views"] = g_views

        def emit_compute_b(ctx_sb):
            """w + weighted reduces + correction + output DMA."""
            r0, t_b = ctx_sb["r0"], ctx_sb["t_b"]
            sup = BLOCK * t_b
            te_t = ctx_sb["te_t"]
            g_views = ctx_sb["g_views"]
            te3 = te_t.rearrange("p (t u) -> p t u", t=t_b)

            # w[i] = T_i - T_{i+1}  (fp32 compute, bf16 out)
            if "w" in skip:
                w_t = te_t
                w_block = lambda t: w_t[:, t * U : t * U + S]
            else:
                w_t = p_mid.tile([BLOCK, S * t_b], wg_dt, tag="w")
                w3 = w_t.rearrange("p (t s) -> p t s", t=t_b)
                nc.vector.tensor_sub(w3, te3[:, :, 0:S], te3[:, :, 1 : S + 1])
                w_block = lambda t: w_t[:, t * S : (t + 1) * S]

            out_t = p_out.tile([BLOCK, 3 * t_b], F32, tag="out")
            if "stt" in skip:
                nc.vector.memset(out_t[:], 0.0)
            else:
                for t in range(t_b):
                    for c in range(3):
                        acc = out_t[:, t * 3 + c : t * 3 + c + 1]
                        eng = nc.vector if c < 3 - gpsimd_channels else nc.gpsimd
                        tag = "scr" if c < 3 - gpsimd_channels else "scrg"
                        scr = p_scr.tile([BLOCK, S], wg_dt, tag=tag)
                        eng.scalar_tensor_tensor(
                            out=scr[:],
                            in0=w_block(t),
                            scalar=0.0,
                            in1=g_views[c][:, t],
                            op0=mybir.AluOpType.bypass,
                            op1=mybir.AluOpType.mult,
                            accum_out=acc,
                        )
                if use_tanh and "sigmoid" not in skip and "w" not in skip:
                    # out = 0.5*sum(w*tanh) + 0.5*sum(w); sum_s w = 1 - T_128
                    halfte = p_scr.tile([BLOCK, t_b], F32, tag="halfte")
                    h1 = halfte.rearrange("p (t o) -> p t o", o=1)
                    nc.vector.tensor_scalar(
                        out=h1,
                        in0=te3[:, :, S : S + 1],
                        scalar1=-0.5,
                        scalar2=0.5,
                        op0=mybir.AluOpType.mult,
                        op1=mybir.AluOpType.add,
                    )
                    o3 = out_t.rearrange("p (t c) -> p t c", c=3)
                    _, h3 = bass.broadcast_tensor_aps(o3, h1)
                    nc.vector.scalar_tensor_tensor(
                        out=o3,
                        in0=o3,
                        scalar=0.5,
                        in1=h3,
                        op0=mybir.AluOpType.mult,
                        op1=mybir.AluOpType.add,
                    )
            if flat_dma:
                eng_of(out_engine).dma_start(
                    out=out_ap[r0 : r0 + sup].rearrange(
                        "(p t) c -> p (t c)", p=BLOCK
                    ),
                    in_=out_t[:],
                )
            else:
                eng_of(out_engine).dma_start(
                    out=out_ap[r0 : r0 + sup].rearrange(
                        "(p t) c -> p t c", p=BLOCK
                    ),
                    in_=out_t.rearrange("p (t c) -> p t c", c=3),
                )

        def emit_all():
            sched = list(t_schedule) if t_schedule else [T] * n_super
            assert sum(sched) * BLOCK == n_rays, (sched, n_rays)
            starts = []
            r0 = 0
            for t_b in sched:
                assert t_b <= T
                starts.append((r0, t_b))
                r0 += BLOCK * t_b
            for _rep in range(repeat):
                if not pipelined:
                    for r0, t_b in starts:
                        c = emit_dma(r0, t_b)
                        emit_compute_a(c)
                        emit_compute_b(c)
                    continue
                # software pipeline: DMA two ahead, phase-A one ahead
                n = len(starts)
                ctxs: dict = {}
                ctxs[0] = emit_dma(*starts[0])
                if n > 1:
                    ctxs[1] = emit_dma(*starts[1])
                emit_compute_a(ctxs[0])
                for k in range(1, n):
                    if k + 1 < n:
                        ctxs[k + 1] = emit_dma(*starts[k + 1])
                    emit_compute_a(ctxs[k])
                    emit_compute_b(ctxs[k - 1])
                    del ctxs[k - 1]
                emit_compute_b(ctxs[n - 1])

        if loop_iters:
            with tc.For_i(0, loop_iters, 1) as _i:
                emit_all()
        else:
            emit_all()
    nc.compile()
    return nc


_NC_CACHE: dict = {}


def _get_nc():
    if "nc" not in _NC_CACHE:
        _NC_CACHE["nc"] = build_nerf_v2()
    return _NC_CACHE["nc"]


def kernel(rgbo: np.ndarray, depth: np.ndarray, **run_kwargs) -> np.ndarray:
    rgbo = np.ascontiguousarray(rgbo, dtype=np.float32)
    depth = np.ascontiguousarray(depth, dtype=np.float32)
    assert rgbo.shape == (N_RAYS, S, 4) and depth.shape == (N_RAYS, S)

    nc = _get_nc()
    in_maps = []
    for i in range(N_CORES):
        sl = slice(i * NC_RAYS, (i + 1) * NC_RAYS)
        in_maps.append({"rgbo": rgbo[sl], "depth": depth[sl]})
    res = run_bass_kernel_spmd(nc, in_maps, core_ids=list(range(N_CORES)), **run_kwargs)
    out = np.concatenate([r["out"] for r in res.results], axis=0)
    if run_kwargs:
        kernel.last_results = res  # stash for profiling harnesses
    return out


# revision 39
# speedup vs baseline: 1.2845x; 1.2845x over previous
"""NeRF volumetric alpha-compositing kernel for Trainium2 (Bass/Tile).

Full inputs:  rgbo [131072, 128, 4] f32, depth [131072, 128] f32.
Full output:  [131072, 3] f32.

Sharding: data-parallel over rays, 8 cores x 16384 rays.

Per-core layout: ray-per-partition; each superblock covers BLOCK=128
partitions x t_b rays per partition (t_b blocks of S=128 samples on the
free dim).  Per superblock:

  delta[s]  = depth[s+1] - depth[s]; delta[S-1]=1e9  (DVE sub + ACT const)
  m[s]      = opacity[s] * delta[s]                  (DVE, one full-width TT)
  cs        = per-ray inclusive cumsum of m          (one DVE scan with a
              0/1 reset pattern: state = A*state + m)
  te[t,0]=1; te[t,1+s] = exp(-cs[t,s])               (ScalarE Exp, fp32;
              the "1" column is written by an ACT Copy, not a DVE memset)
  w[i]      = te[i] - te[i+1]   -> bf16              (DVE; = T_i * alpha_i)
  g_c       = sigmoid(rgb_c)    -> bf16              (ONE ScalarE activation
              over all 3 channels into dense per-channel planes)
  wg_c      = w * g_c           (DVE TT, bf16 2x mode)
  out[t,c]  = sum_s wg_c        (DVE grouped tensor_reduce, axis=X)

The last-sample FAR_DELTA=1e9 is exact: te[t,S]=exp(-cs[S-1]) underflows
to 0 whenever opacity[S-1] > ~1e-7, else matches the reference.

Measured on HW (loop_iters slope method; absolute numbers drift +-25%
across sessions with shared-tenancy load, so A/Bs were bracketed
within one process): this config beat the previous-best stt-based
kernel by ~6 us and the original baseline by ~10-15%; DMA-only floor
is 113.8 us (42 MB/core at ~370 GB/s).  Verified-slower alternatives:
GPSIMD compute (DVE perf modes lock it out of SBUF, +50 us), software-
pipelined emission, bufs=3, end-tapered schedules, bf16 scan I/O, and
the tanh activation-table trick.
"""

from contextlib import ExitStack

import numpy as np

import concourse.bass as bass
import concourse.tile as tile
from concourse import bacc, mybir
from concourse.bass_utils import run_bass_kernel_spmd

N_RAYS = 131072
S = 128
N_CORES = 8
NC_RAYS = N_RAYS // N_CORES  # 16384 rays per core
BLOCK = 128                  # rays per partition-block
F32 = mybir.dt.float32
BF16 = mybir.dt.bfloat16


def build_nerf_v2(
    n_rays: int = NC_RAYS,
    t_blocks: int = 8,
    use_bf16: bool = True,
    merged_scan: bool = True,
    gpsimd_delta_m: bool = False,
    gpsimd_channels: int = 0,
    flat_dma: bool = False,
    depth_engine: str = "scalar",
    out_engine: str = "scalar",
    use_tanh: bool = False,
    pipelined: bool = False,
    grouped_reduce: bool = True,
    fused_sigmoid: bool = True,
    act_memset: bool = True,
    fold_tail: bool = True,
    scan_bf16: bool = False,
    t_schedule: tuple | None = None,
    repeat: int = 1,
    loop_iters: int = 0,
    skip: tuple = (),
    bufs: int = 2,
    scr_bufs: int = 4,
) -> bass.Bass:
    T = t_blocks
    SUPER = BLOCK * T
    n_super = n_rays // SUPER
    U = S + 1  # te-table stride: 129 entries (T_0..T_128)
    wg_dt = BF16 if use_bf16 else F32
    m_dt = BF16 if scan_bf16 else F32

    nc = bacc.Bacc("TRN2", target_bir_lowering=False, debug=False)
    rgbo_h = nc.declare_dram_parameter("rgbo", [n_rays, S, 4], F32, isOutput=False)
    depth_h = nc.declare_dram_parameter("depth", [n_rays, S], F32, isOutput=False)
    out_h = nc.declare_dram_parameter("out", [n_rays, 3], F32, isOutput=True)

    rgbo_ap = rgbo_h.ap()
    depth_ap = depth_h.ap()
    out_ap = out_h.ap()

    def eng_of(name):
        return {"sync": nc.sync, "scalar": nc.scalar, "gpsimd": nc.gpsimd}[name]

    with ExitStack() as ctx:
        tc = ctx.enter_context(tile.TileContext(nc))
        p_rgbo = ctx.enter_context(tc.tile_pool(name="rgbo", bufs=bufs))
        p_depth = ctx.enter_context(tc.tile_pool(name="depth", bufs=bufs))
        p_g = ctx.enter_context(tc.tile_pool(name="g", bufs=bufs))
        p_mid = ctx.enter_context(tc.tile_pool(name="mid", bufs=bufs))
        p_scr = ctx.enter_context(tc.tile_pool(name="scr", bufs=scr_bufs))
        p_out = ctx.enter_context(tc.tile_pool(name="outp", bufs=bufs))
        p_const = ctx.enter_context(tc.tile_pool(name="const", bufs=1))

        # reset pattern for the merged scan: 0.0 at each ray's first
        # sample, 1.0 elsewhere  (state = A*state + m)
        a_t = None
        if merged_scan:
            a_t = p_const.tile([BLOCK, S * T], m_dt, tag="a")
            nc.vector.memset(a_t[:], 1.0)
            nc.vector.memset(
                a_t.rearrange("p (t s) -> p t s", t=T)[:, :, 0:1], 0.0
            )

        def emit_dma(r0, t_b):
            """Issue the input DMAs for one superblock; returns tile ctx."""
            sup = BLOCK * t_b
            rgbo_t = p_rgbo.tile([BLOCK, 4 * S * t_b], F32, tag="rgbo")
            depth_t = p_depth.tile([BLOCK, S * t_b], F32, tag="depth")
            if flat_dma:
                # partition p holds rays [p*t_b, (p+1)*t_b), contiguous in
                # DRAM -> one descriptor line per partition
                rgbo_dst = rgbo_t[:]
                rgbo_src = rgbo_ap[r0 : r0 + sup].rearrange(
                    "(p t) s c -> p (t s c)", p=BLOCK
                )
                depth_dst = depth_t[:]
                depth_src = depth_ap[r0 : r0 + sup].rearrange(
                    "(p t) s -> p (t s)", p=BLOCK
                )
            else:
                rgbo_dst = rgbo_t.rearrange("p (t f) -> p t f", t=t_b)
                rgbo_src = rgbo_ap[r0 : r0 + sup].rearrange(
                    "(p t) s c -> p t (s c)", p=BLOCK
                )
                depth_dst = depth_t.rearrange("p (t s) -> p t s", t=t_b)
                depth_src = depth_ap[r0 : r0 + sup].rearrange(
                    "(p t) s -> p t s", p=BLOCK
                )
            nc.sync.dma_start(out=rgbo_dst, in_=rgbo_src)
            eng_of(depth_engine).dma_start(out=depth_dst, in_=depth_src)
            return {"r0": r0, "t_b": t_b, "rgbo_t": rgbo_t, "depth_t": depth_t}

        def emit_compute_a(ctx_sb):
            """delta/m/scan/exp/tanh for one superblock (extends ctx)."""
            r0, t_b = ctx_sb["r0"], ctx_sb["t_b"]
            rgbo_t, depth_t = ctx_sb["rgbo_t"], ctx_sb["depth_t"]
            rgbo4 = rgbo_t.rearrange("p (t s c) -> p t s c", t=t_b, s=S, c=4)
            depth3 = depth_t.rearrange("p (t s) -> p t s", t=t_b)

            # per-channel color nonlinearity -> dense bf16 tiles (ScalarE)
            act_fn = (
                mybir.ActivationFunctionType.Tanh
                if use_tanh
                else mybir.ActivationFunctionType.Sigmoid
            )
            act_scale = 0.5 if use_tanh else 1.0
            if "sigmoid" in skip:
                g_views = [rgbo4[:, :, :, c] for c in range(3)]
            elif fused_sigmoid:
                # one activation over all 3 channels into dense per-channel
                # planes: in [p, c, t, s] (strides 1, 512, 4), out [p,(c t s)]
                g3_t = p_g.tile([BLOCK, 3 * S * t_b], wg_dt, tag="g3")
                in_ap = rgbo_t[:]
                in4 = bass.AP(
                    in_ap.tensor,
                    in_ap.offset,
                    [in_ap.ap[0], [1, 3], [4 * S, t_b], [4, S]],
                )
                nc.scalar.activation(
                    g3_t.rearrange("p (c t s) -> p c t s", c=3, t=t_b),
                    in4,
                    act_fn,
                    scale=act_scale,
                )
                g_views = [
                    g3_t[:, c * S * t_b : (c + 1) * S * t_b].rearrange(
                        "p (t s) -> p t s", t=t_b
                    )
                    for c in range(3)
                ]
            else:
                g_views = []
                for c in range(3):
                    g_c = p_g.tile([BLOCK, S * t_b], wg_dt, tag=f"g{c}")
                    nc.scalar.activation(
                        g_c.rearrange("p (t s) -> p t s", t=t_b),
                        rgbo4[:, :, :, c],
                        act_fn,
                        scale=act_scale,
                    )
                    g_views.append(g_c.rearrange("p (t s) -> p t s", t=t_b))

            # delta, m on GPSIMD (or DVE)
            eng_dm = nc.gpsimd if gpsimd_delta_m else nc.vector
            if "dm" in skip:
                m_t = depth_t
            else:
                delta_t = p_mid.tile([BLOCK, S * t_b], F32, tag="delta")
                delta3 = delta_t.rearrange("p (t s) -> p t s", t=t_b)
                eng_dm.tensor_sub(
                    delta3[:, :, 0 : S - 1],
                    depth3[:, :, 1:S],
                    depth3[:, :, 0 : S - 1],
                )
                m_t = p_mid.tile([BLOCK, S * t_b], m_dt, tag="m")
                m3 = m_t.rearrange("p (t s) -> p t s", t=t_b)
                if fold_tail:
                    # delta[S-1] = 1e9 written on ACT, then one full-width
                    # multiply covers the FAR tail too
                    nc.scalar.activation(
                        delta3[:, :, S - 1 : S],
                        depth3[:, :, S - 1 : S],
                        mybir.ActivationFunctionType.Copy,
                        bias=1.0e9,
                        scale=0.0,
                    )
                    eng_dm.tensor_mul(m3, delta3, rgbo4[:, :, :, 3])
                else:
                    eng_dm.tensor_mul(
                        m3[:, :, 0 : S - 1],
                        delta3[:, :, 0 : S - 1],
                        rgbo4[:, :, 0 : S - 1, 3],
                    )
                    eng_dm.tensor_scalar_mul(
                        m3[:, :, S - 1], rgbo4[:, :, S - 1, 3], 1.0e9
                    )

            # per-ray inclusive cumsum (DVE scan)
            if "scan" in skip:
                cs_t = m_t
            elif merged_scan:
                cs_t = p_mid.tile([BLOCK, S * t_b], m_dt, tag="cs")
                nc.vector.tensor_tensor_scan(
                    cs_t[:],
                    a_t[:, 0 : S * t_b],
                    m_t[:],
                    0.0,
                    mybir.AluOpType.mult,
                    mybir.AluOpType.add,
                )
            else:
                cs_t = p_mid.tile([BLOCK, S * t_b], F32, tag="cs")
                for t in range(t_b):
                    nc.vector.tensor_tensor_scan(
                        cs_t[:, t * S : (t + 1) * S],
                        m_t[:, t * S : (t + 1) * S],
                        m_t[:, t * S : (t + 1) * S],
                        0.0,
                        mybir.AluOpType.add,
                        mybir.AluOpType.bypass,
                    )

            # te table (fp32): te[t,0]=1, te[t,1+s]=exp(-cs[t,s])
            te_t = p_mid.tile([BLOCK, U * t_b], F32, tag="te")
            te3 = te_t.rearrange("p (t u) -> p t u", t=t_b)
            if act_memset:
                nc.scalar.activation(
                    te3[:, :, 0:1],
                    cs_t.rearrange("p (t s) -> p t s", t=t_b)[:, :, 0:1],
                    mybir.ActivationFunctionType.Copy,
                    bias=1.0,
                    scale=0.0,
                )
            else:
                nc.vector.memset(te3[:, :, 0:1], 1.0)
            nc.scalar.activation(
                te3[:, :, 1 : S + 1],
                cs_t.rearrange("p (t s) -> p t s", t=t_b),
                mybir.ActivationFunctionType.Exp,
                scale=-1.0,
            )
            ctx_sb["te_t"] = te_t
            ctx_sb["g_views"] = g_views

        def emit_compute_b(ctx_sb):
            """w + weighted reduces + correction + output DMA."""
            r0, t_b = ctx_sb["r0"], ctx_sb["t_b"]
            sup = BLOCK * t_b
            te_t = ctx_sb["te_t"]
            g_views = ctx_sb["g_views"]
            te3 = te_t.rearrange("p (t u) -> p t u", t=t_b)

            # w[i] = T_i - T_{i+1}  (fp32 compute, bf16 out)
            if "w" in skip:
                w_t = te_t
                w_block = lambda t: w_t[:, t * U : t * U + S]
            else:
                w_t = p_mid.tile([BLOCK, S * t_b], wg_dt, tag="w")
                w3 = w_t.rearrange("p (t s) -> p t s", t=t_b)
                nc.vector.tensor_sub(w3, te3[:, :, 0:S], te3[:, :, 1 : S + 1])
                w_block = lambda t: w_t[:, t * S : (t + 1) * S]

            out_t = p_out.tile([BLOCK, 3 * t_b], F32, tag="out")
            o3 = out_t.rearrange("p (t c) -> p t c", c=3)
            if "stt" in skip:
                nc.vector.memset(out_t[:], 0.0)
            elif grouped_reduce:
                # per channel: one full-width bf16 2x multiply + one grouped
                # reduce (innermost axis) -> 6 DVE instrs instead of 24
                assert "w" not in skip
                for c in range(3):
                    wg_c = p_scr.tile([BLOCK, S * t_b], wg_dt, tag=f"wg{c}")
                    wg3 = wg_c.rearrange("p (t s) -> p t s", t=t_b)
                    nc.vector.tensor_mul(wg3, w3, g_views[c])
                    nc.vector.tensor_reduce(
                        o3[:, :, c : c + 1],
                        wg3,
                        mybir.AxisListType.X,
                        mybir.AluOpType.add,
                    )
            else:
                for t in range(t_b):
                    for c in range(3):
                        acc = out_t[:, t * 3 + c : t * 3 + c + 1]
                        eng = nc.vector if c < 3 - gpsimd_channels else nc.gpsimd
                        tag = "scr" if c < 3 - gpsimd_channels else "scrg"
                        scr = p_scr.tile([BLOCK, S], wg_dt, tag=tag)
                        eng.scalar_tensor_tensor(
                            out=scr[:],
                            in0=w_block(t),
                            scalar=0.0,
                            in1=g_views[c][:, t],
                            op0=mybir.AluOpType.bypass,
                            op1=mybir.AluOpType.mult,
                            accum_out=acc,
                        )
                if use_tanh and "sigmoid" not in skip and "w" not in skip:
                    # out = 0.5*sum(w*tanh) + 0.5*sum(w); sum_s w = 1 - T_128
                    halfte = p_scr.tile([BLOCK, t_b], F32, tag="halfte")
                    h1 = halfte.rearrange("p (t o) -> p t o", o=1)
                    nc.vector.tensor_scalar(
                        out=h1,
                        in0=te3[:, :, S : S + 1],
                        scalar1=-0.5,
                        scalar2=0.5,
                        op0=mybir.AluOpType.mult,
                        op1=mybir.AluOpType.add,
                    )
                    o3 = out_t.rearrange("p (t c) -> p t c", c=3)
                    _, h3 = bass.broadcast_tensor_aps(o3, h1)
                    nc.vector.scalar_tensor_tensor(
                        out=o3,
                        in0=o3,
                        scalar=0.5,
                        in1=h3,
                        op0=mybir.AluOpType.mult,
                        op1=mybir.AluOpType.add,
                    )
            if flat_dma:
                eng_of(out_engine).dma_start(
                    out=out_ap[r0 : r0 + sup].rearrange(
                        "(p t) c -> p (t c)", p=BLOCK
                    ),
                    in_=out_t[:],
                )
            else:
                eng_of(out_engine).dma_start(
                    out=out_ap[r0 : r0 + sup].rearrange(
                        "(p t) c -> p t c", p=BLOCK
                    ),
                    in_=out_t.rearrange("p (t c) -> p t c", c=3),
                )

        def emit_all():
            sched = list(t_schedule) if t_schedule else [T] * n_super
            assert sum(sched) * BLOCK == n_rays, (sched, n_rays)
            starts = []
            r0 = 0
            for t_b in sched:
                assert t_b <= T
                starts.append((r0, t_b))
                r0 += BLOCK * t_b
            for _rep in range(repeat):
                if not pipelined:
                    for r0, t_b in starts:
                        c = emit_dma(r0, t_b)
                        emit_compute_a(c)
                        emit_compute_b(c)
                    continue
                # software pipeline: DMA two ahead, phase-A one ahead
                n = len(starts)
                ctxs: dict = {}
                ctxs[0] = emit_dma(*starts[0])
                if n > 1:
                    ctxs[1] = emit_dma(*starts[1])
                emit_compute_a(ctxs[0])
                for k in range(1, n):
                    if k + 1 < n:
                        ctxs[k + 1] = emit_dma(*starts[k + 1])
                    emit_compute_a(ctxs[k])
                    emit_compute_b(ctxs[k - 1])
                    del ctxs[k - 1]
                emit_compute_b(ctxs[n - 1])

        if loop_iters:
            with tc.For_i(0, loop_iters, 1) as _i:
                emit_all()
        else:
            emit_all()
    nc.compile()
    return nc


_NC_CACHE: dict = {}


def _get_nc():
    if "nc" not in _NC_CACHE:
        _NC_CACHE["nc"] = build_nerf_v2()
    return _NC_CACHE["nc"]


def kernel(rgbo: np.ndarray, depth: np.ndarray, **run_kwargs) -> np.ndarray:
    rgbo = np.ascontiguousarray(rgbo, dtype=np.float32)
    depth = np.ascontiguousarray(depth, dtype=np.float32)
    assert rgbo.shape == (N_RAYS, S, 4) and depth.shape == (N_RAYS, S)

    nc = _get_nc()
    in_maps = []
    for i in range(N_CORES):
        sl = slice(i * NC_RAYS, (i + 1) * NC_RAYS)
        in_maps.append({"rgbo": rgbo[sl], "depth": depth[sl]})
    res = run_bass_kernel_spmd(nc, in_maps, core_ids=list(range(N_CORES)), **run_kwargs)
    out = np.concatenate([r["out"] for r in res.results], axis=0)
    if run_kwargs:
        kernel.last_results = res  # stash for profiling harnesses
    return out
